# revision 1
# baseline (speedup 1.0000x reference)
"""Multi-head self-attention (B=8, N=1024, C=768, H=12) on 8 Trainium2 cores.

Strategy: data parallel — one batch element per NeuronCore, no collectives.

Per-core program (x_b is [N, C], shipped pre-transposed as xT [C, N]):
  1. qkT  [o, n] = wqkvT[:, o].T @ xT          o in [0, 1536)   (q and k, transposed)
       q rows evicted with  *SCALE and +SCALE*bq  (k bias cancels in softmax)
  2. v    [n, o] = xT[:, n].T @ wqkvT[:, 1536:] (natural layout), evicted into
       vext [n, 12*65] with a ones column appended per head
  3. per head h:  S.T[m, n] = kT_h.T @ qT_h    (K=64 matmul)
       E = exp(S.T)  (ACT, no max subtraction: logits ~ N(0,1))
       U [65, n] = vext_h.T @ E  — rows 0..63 = unnormalized out.T, row 64 = sum
       Z row moved to partition 0 by DMA; recip (DVE); gpsimd partition_broadcast
       outcT_h [d, n] = U[0:64] * recip  (DVE)
  4. final [n, co] = outcT[:, n].T @ wpT + pbe   (pbe = proj_b + bv @ proj_w.T)

Emission order interleaves phases so ACT (exp) overlaps PE throughout:
  qk(t=0) -> QK(0) QK(1) -> v -> { qk(t) ; AV(2t-2)||QK(2t) ; AV(2t-1)||QK(2t+1) }
  -> AV(10) AV(11) -> proj

MM_MODE "f32r" streams fp32 at 1 cycle/row (PE rounds internally);
"f32" is exact fp32 at 4 cycles/row.
"""

import os
from contextlib import ExitStack

import numpy as np

B, N, C = 8, 1024, 768
H, D = 12, 64
SCALE = D**-0.5
NCORES = 8

P = 128
CC = C // P        # 6  c-chunks
NT = N // P        # 8  n-chunks of 128
VW = H * (D + 1)   # 780: v + ones column per head

MM_MODE = os.environ.get("BASS_MM", "f32r")
PT_BUFS = int(os.environ.get("BASS_PT_BUFS", "17"))
DEBUG = bool(int(os.environ.get("BASS_DEBUG", "0")))
TRUNC = os.environ.get("BASS_TRUNC", "")
INTERLEAVE = bool(int(os.environ.get("BASS_INTERLEAVE", "1")))

LAST_RESULTS = None  # BassKernelResults of the most recent run (for test.py)
LAST_IN_MAPS = None

_built = {}


class _nullpool:
    def __enter__(self):
        return None

    def __exit__(self, *a):
        return False


def _build():
    import concourse.bass as bass
    import concourse.mybir as mybir
    import concourse.tile as tile
    from concourse import bacc

    f32 = mybir.dt.float32
    fmm = mybir.dt.float32r if MM_MODE == "f32r" else f32
    AF = mybir.ActivationFunctionType
    ALU = mybir.AluOpType

    nc = bacc.Bacc("TRN2", target_bir_lowering=False, debug=False, num_devices=NCORES)

    xT_d = nc.dram_tensor("xT", [C, N], fmm, kind="ExternalInput").ap()
    wqkr_d = nc.dram_tensor("wqkr", [C, 2 * C], fmm, kind="ExternalInput").ap()
    wv_d = nc.dram_tensor("wv", [C, C], fmm, kind="ExternalInput").ap()
    wpT_d = nc.dram_tensor("wpT", [C, C], fmm, kind="ExternalInput").ap()
    bq_d = nc.dram_tensor("bq", [P, CC], f32, kind="ExternalInput").ap()
    pbe_d = nc.dram_tensor("pbe", [P, C], f32, kind="ExternalInput").ap()
    out_d = nc.dram_tensor("out", [N, C], f32, kind="ExternalOutput").ap()
    trunc_u = (
        nc.dram_tensor("trunc_u", [D + 1, N], f32, kind="ExternalOutput").ap()
        if TRUNC == "av0"
        else None
    )
    trunc_oc = (
        nc.dram_tensor("trunc_oc", [P, N], f32, kind="ExternalOutput").ap()
        if TRUNC in ("tail0", "avall")
        else None
    )
    trunc_hit = [False]
    dbg = {}
    if DEBUG:
        for nm, shp in [("dbg_qT", [P, N]), ("dbg_kT", [P, N]), ("dbg_vext", [P, VW]),
                        ("dbg_pt", [P, N]), ("dbg_u", [D + 1, 512]), ("dbg_psr", [D, 512])] + [
                        (f"dbg_outcT{i}", [P, N]) for i in range(CC)] + [
                        (f"dbg_u{h}", [D + 1, 512]) for h in range(H)]:
            dbg[nm] = nc.dram_tensor(nm, shp, f32, kind="ExternalOutput").ap()

    with tile.TileContext(nc) as tc, ExitStack() as ctx:
        persist = ctx.enter_context(tc.tile_pool(name="persist", bufs=1))
        qkpool = ctx.enter_context(tc.tile_pool(name="qkpool", bufs=1))
        rpool = ctx.enter_context(tc.tile_pool(name="rpool", bufs=3))
        ppool = ctx.enter_context(tc.tile_pool(name="ppool", bufs=PT_BUFS))
        wqkt_pool = ctx.enter_context(tc.tile_pool(name="wqkt", bufs=2))
        ocpool = ctx.enter_context(tc.tile_pool(name="ocpool", bufs=1))
        ps2 = ctx.enter_context(tc.tile_pool(name="ps2", bufs=3, space="PSUM"))
        psav = ctx.enter_context(tc.tile_pool(name="psav", bufs=2, space="PSUM"))

        vext = [
            persist.tile([P, VW], fmm, name=f"vext{i}", tag=f"vext{i}")
            for i in range(NT)
        ]
        bq_t = persist.tile([P, CC], f32, name="bq_t", tag="bq_t")
        dbg_psr_sb = persist.tile([D, 512], f32, name="dbg_psr_sb", tag="dbg_psr_sb")
        outcT = [
            ocpool.tile([P, N], fmm, name=f"outcT{i}", tag=f"outcT{i}")
            for i in range(CC)
        ]

        nc.sync.dma_start(bq_t[:], bq_d[:])
        for i in range(NT):
            nc.vector.memset(
                vext[i].bitcast(f32).rearrange("p (h e) -> p h e", e=D + 1)[
                    :, :, D : D + 1
                ],
                1.0,
            )

        pT = {}  # (h, mc) -> tile

        def load_wq(t):
            wq_t = [
                wqkt_pool.tile([P, 2 * P], fmm, name="wq_t", tag=f"wq{c}")
                for c in range(CC)
            ]
            for c in range(CC):
                nc.sync.dma_start(
                    wq_t[c][:],
                    wqkr_d[c * P : (c + 1) * P, t * 2 * P : (t + 1) * 2 * P],
                )
            return wq_t

        def emit_qk(t, x_t, wq_t=None):
            """Produce qT/kT ring tiles for heads 2t, 2t+1 (streams weights)."""
            if wq_t is None:
                wq_t = load_wq(t)
            qT_t = qkpool.tile([P, N], fmm, name="qT_t", tag="qT_t")
            kT_t = qkpool.tile([P, N], fmm, name="kT_t", tag="kT_t")
            pss = []
            for wofs in (0, P):
                ps = ps2.tile([P, N], f32, name="ps", tag="ps")
                for nh in range(2):
                    for c in range(CC):
                        nc.tensor.matmul(
                            ps[:, nh * 512 : (nh + 1) * 512],
                            wq_t[c][:, wofs : wofs + P],
                            x_t[c][:, nh * 512 : (nh + 1) * 512],
                            start=(c == 0),
                            stop=(c == CC - 1),
                        )
                pss.append(ps)
            # interleave half-evictions so QK can start after the first halves
            for half in range(2):
                s = slice(half * 512, (half + 1) * 512)
                nc.vector.tensor_scalar(
                    out=qT_t[:, s],
                    in0=pss[0][:, s],
                    scalar1=SCALE,
                    scalar2=bq_t[:, t : t + 1],
                    op0=ALU.mult,
                    op1=ALU.add,
                )
                nc.vector.tensor_copy(kT_t[:, s], pss[1][:, s])
            return qT_t, kT_t

        def emit_QK_mc(h, mc, qT_t, kT_t):
            """One m-chunk of scores for head h: S.T[mc-block, :] -> exp -> pT."""
            r0 = (h % 2) * D
            ps = ps2.tile([P, N], f32, name="ps", tag="ps")
            for nh in range(2):
                nc.tensor.matmul(
                    ps[:, nh * 512 : (nh + 1) * 512],
                    kT_t[r0 : r0 + D, mc * P : (mc + 1) * P],
                    qT_t[r0 : r0 + D, nh * 512 : (nh + 1) * 512],
                    start=True,
                    stop=True,
                )
            pt = ppool.tile([P, N], fmm, name="pt", tag="pt")
            nc.scalar.activation(pt[:], ps[:], AF.Exp)
            pT[(h, mc)] = pt

        def emit_AV(h, interleave=None):
            """Attention @ V for head h; optionally interleave QK m-chunks of
            a later head (deps: its qT/kT tiles) between AV accumulation steps."""
            psa = [
                psav.tile([D + 1, 512], f32, name="psa", tag="psa") for _ in range(2)
            ]
            for mc in range(NT):
                for nh in range(2):
                    nc.tensor.matmul(
                        psa[nh][:],
                        vext[mc][:, h * (D + 1) : (h + 1) * (D + 1)],
                        pT[(h, mc)][:, nh * 512 : (nh + 1) * 512],
                        start=(mc == 0),
                        stop=(mc == NT - 1),
                    )
                if interleave is not None:
                    h2, qT_t, kT_t = interleave
                    emit_QK_mc(h2, mc, qT_t, kT_t)
            if TRUNC == "tail0" and h == 1:
                # let heads 0,1 complete fully, then dump outcT[0]
                pass
            if TRUNC == "av0" and h == 0:
                for nh in range(2):
                    tcp = rpool.tile([D + 1, 512], f32, name="tcp", tag="u_sb")
                    nc.vector.tensor_copy(tcp[:], psa[nh][:])
                    nc.sync.dma_start(trunc_u[:, nh * 512 : (nh + 1) * 512], tcp[:])
                trunc_hit[0] = True
                return
            for mc in range(NT):
                del pT[(h, mc)]
            ti, r0 = h // 2, (h % 2) * D
            for nh in range(2):
                u_sb = rpool.tile([D + 1, 512], f32, name="u_sb", tag="u_sb")
                nc.vector.tensor_copy(u_sb[:], psa[nh][:])  # frees the PSUM bank
                zr = rpool.tile([1, 512], f32, name="zr", tag="zr")
                nc.sync.dma_start(zr[:], u_sb[D : D + 1, :])  # Z row -> partition 0
                rc = rpool.tile([1, 512], f32, name="rc", tag="rc")
                nc.vector.reciprocal(rc[:], zr[:])
                rb = rpool.tile([D, 512], f32, name="rb", tag="rb")
                nc.gpsimd.partition_broadcast(rb[:], rc[:])
                nc.vector.tensor_mul(
                    outcT[ti][r0 : r0 + D, nh * 512 : (nh + 1) * 512],
                    u_sb[0:D, :],
                    rb[:],
                )

        # ---------------- emission ----------------
        with tc.tile_pool(name="xw", bufs=1) as xw:
            wq0 = load_wq(0)
            x_t = [xw.tile([P, N], fmm, name=f"x{i}", tag=f"x{i}") for i in range(CC)]
            for i in range(CC):  # first halves via SWDGE, parallel to HWDGE wq0
                nc.gpsimd.dma_start(x_t[i][:, 0:512], xT_d[i * P : (i + 1) * P, 0:512])
            for i in range(CC):
                nc.sync.dma_start(
                    x_t[i][:, 512:1024], xT_d[i * P : (i + 1) * P, 512:1024]
                )

            with tc.tile_pool(name="wvp", bufs=1) as wvp:
                # qk for heads 0,1 then their scores (feeds ACT during v phase)
                qT_t, kT_t = emit_qk(0, x_t, wq_t=wq0)
                if DEBUG:
                    nc.sync.dma_start(dbg["dbg_qT"][:], qT_t.bitcast(f32)[:])
                    nc.sync.dma_start(dbg["dbg_kT"][:], kT_t.bitcast(f32)[:])
                for mc in range(NT):
                    emit_QK_mc(0, mc, qT_t, kT_t)
                    if DEBUG and mc == 0:
                        nc.sync.dma_start(dbg["dbg_pt"][:], pT[(0, 0)].bitcast(f32)[:])
                for mc in range(NT):
                    emit_QK_mc(1, mc, qT_t, kT_t)

                wv_t = [
                    wvp.tile([P, C], fmm, name=f"wv{i}", tag=f"wv{i}")
                    for i in range(CC)
                ]
                for i in range(CC):
                    nc.sync.dma_start(wv_t[i][:], wv_d[i * P : (i + 1) * P, :])

                # v projection into vext (natural layout + ones columns)
                for nt in range(NT):
                    ps = ps2.tile([P, C], f32, name="ps", tag="ps")
                    for o0, o1 in ((0, 512), (512, 768)):
                        for c in range(CC):
                            nc.tensor.matmul(
                                ps[:, o0:o1],
                                x_t[c][:, nt * P : (nt + 1) * P],
                                wv_t[c][:, o0:o1],
                                start=(c == 0),
                                stop=(c == CC - 1),
                            )
                    dst = vext[nt].rearrange("p (h e) -> p h e", e=D + 1)[:, :, 0:D]
                    nc.vector.tensor_copy(
                        dst, ps[:].rearrange("p (h d) -> p h d", d=D)
                    )
                    if DEBUG and nt == 0:
                        nc.sync.dma_start(dbg["dbg_vext"][:], vext[0].bitcast(f32)[:])

            # steady state: qk(t) ; AV(2t-2) || QK(2t) ; AV(2t-1) || QK(2t+1)
            if trunc_hit[0]:
                pass
            elif INTERLEAVE:
                for t in range(1, CC):
                    qT_n, kT_n = emit_qk(t, x_t)
                    emit_AV(2 * t - 2, interleave=(2 * t, qT_n, kT_n))
                    emit_AV(2 * t - 1, interleave=(2 * t + 1, qT_n, kT_n))
                    qT_t, kT_t = qT_n, kT_n
            else:
                emit_AV(0)
                if not trunc_hit[0]:
                    emit_AV(1)
                for t in range(1 if not trunc_hit[0] else CC, CC):
                    qT_n, kT_n = emit_qk(t, x_t)
                    for hh in (2 * t, 2 * t + 1):
                        for mc in range(NT):
                            emit_QK_mc(hh, mc, qT_n, kT_n)
                        emit_AV(hh)
                    qT_t, kT_t = qT_n, kT_n

        if INTERLEAVE and not trunc_hit[0]:
            emit_AV(2 * CC - 2)
            emit_AV(2 * CC - 1)
        if TRUNC == "avall":
            nc.sync.dma_start(trunc_oc[:], outcT[0].bitcast(f32)[:])
            trunc_hit[0] = True

        if DEBUG:
            for i in range(CC):
                nc.sync.dma_start(dbg[f"dbg_outcT{i}"][:], outcT[i].bitcast(f32)[:])

        # ---------------- output projection ----------------
        with tc.tile_pool(name="projp", bufs=1) as projp, tc.tile_pool(
            name="ostage", bufs=3
        ) as ostage:
            if trunc_hit[0]:
                # still write the declared "out" so the NEFF binds it
                zt = ostage.tile([P, C], f32, name="ot", tag="ot")
                nc.vector.memset(zt[:], 0.0)
                for nt in range(NT):
                    nc.sync.dma_start(out_d[nt * P : (nt + 1) * P, :], zt[:])
            wp_t = [
                projp.tile([P, C], fmm, name=f"wp{i}", tag=f"wp{i}") for i in range(CC)
            ]
            pbe_t = projp.tile([P, C], f32, name="pbe_t", tag="pbe_t")
            for i in range(CC):
                nc.sync.dma_start(wp_t[i][:], wpT_d[i * P : (i + 1) * P, :])
            nc.sync.dma_start(pbe_t[:], pbe_d[:])

            for nt in range(NT) if not trunc_hit[0] else []:
                ps = ps2.tile([P, C], f32, name="ps", tag="ps")
                for o0, o1 in ((0, 512), (512, 768)):
                    for c in range(CC):
                        nc.tensor.matmul(
                            ps[:, o0:o1],
                            outcT[c][:, nt * P : (nt + 1) * P],
                            wp_t[c][:, o0:o1],
                            start=(c == 0),
                            stop=(c == CC - 1),
                        )
                ot = ostage.tile([P, C], f32, name="ot", tag="ot")
                nc.vector.tensor_add(ot[:], ps[:], pbe_t[:])
                nc.sync.dma_start(out_d[nt * P : (nt + 1) * P, :], ot[:])

    nc.compile()
    return nc


def kernel(x, qkv_w, qkv_b, proj_w, proj_b):
    global LAST_RESULTS, LAST_IN_MAPS
    from concourse.bass_utils import run_bass_kernel_spmd

    key = (MM_MODE, PT_BUFS, INTERLEAVE, DEBUG, TRUNC)
    if key not in _built:
        _built[key] = _build()
    nc = _built[key]

    x = np.asarray(x, np.float32)
    qkv_w = np.asarray(qkv_w, np.float32)
    qkv_b = np.asarray(qkv_b, np.float32)
    proj_w = np.asarray(proj_w, np.float32)
    proj_b = np.asarray(proj_b, np.float32)

    wT = np.ascontiguousarray(qkv_w.T)  # [C, 3C]
    # per-t interleave: block t = [q cols t*128:(t+1)*128 | k cols same range]
    wqkr = np.concatenate(
        [
            np.concatenate((wT[:, t * P : (t + 1) * P], wT[:, C + t * P : C + (t + 1) * P]), axis=1)
            for t in range(CC)
        ],
        axis=1,
    )
    wqkr = np.ascontiguousarray(wqkr)
    wv = np.ascontiguousarray(wT[:, 2 * C :])
    wpT = np.ascontiguousarray(proj_w.T)
    bq = np.ascontiguousarray((SCALE * qkv_b[:C]).reshape(CC, P).T)
    pbe = proj_b + qkv_b[2 * C :] @ proj_w.T
    pbe_b = np.ascontiguousarray(np.broadcast_to(pbe, (P, C)))

    in_maps = [
        {
            "xT": np.ascontiguousarray(x[b].T),
            "wqkr": wqkr,
            "wv": wv,
            "wpT": wpT,
            "bq": bq,
            "pbe": pbe_b,
        }
        for b in range(B)
    ]

    LAST_IN_MAPS = in_maps
    trace = bool(int(os.environ.get("BASS_PROFILE", "0")))
    res = run_bass_kernel_spmd(nc, in_maps, list(range(NCORES)), trace=trace)
    LAST_RESULTS = res
    return np.stack([res.results[b]["out"] for b in range(B)])



# revision 4
# speedup vs baseline: 1.0560x; 1.0560x over previous
"""Multi-head self-attention (B=8, N=1024, C=768, H=12) on 8 Trainium2 cores.

Strategy: data parallel — one batch element per NeuronCore, no collectives.

Per-core program (x_b is [N, C], shipped pre-transposed as xT [C, N], all
matmul operands in bf16, PSUM accumulation in fp32):
  1. qkT  [o, n] = wqk[:, o].T @ xT            o in [0, 1536)   (q and k, transposed)
       q rows evicted with  *SCALE and +SCALE*bq  (k bias cancels in softmax)
  2. v    [n, o] = xT[:, n].T @ wv             (natural layout), evicted into
       vext [n, 12*65] with a ones column appended per head
  3. per head h:  S.T[m, n] = kT_h.T @ qT_h    (K=64 matmul)
       E = exp(S.T)  (ACT, no max subtraction: logits ~ N(0,1))
       U [65, n] = vext_h.T @ E  — rows 0..63 = unnormalized out.T, row 64 = Z
       Z row (PSUM partition 64) partition-broadcast (gpsimd) to [64, n],
       outcT_h [d, n] = U[0:64] / Zb  (DVE divide, straight from PSUM)
  4. final [n, co] = outcT[:, n].T @ wpT + pbe   (pbe = proj_b + bv @ proj_w.T)

Schedule (engine-overlap driven; PE is the bottleneck at ~146 us busy):
  startup   batched multi-dim DMAs, ordered so the first qk matmuls start ~2us
  prelude   qk(0) -> per mc: QK(0,mc); v-legA(mc); QK(1,mc); v-legB(mc)
            (v-projection matmuls fill the PE while ACT drains the score exps)
  steady    t=1..5: qk(t) ; AV(2t-2)+QK(2t) ; AV(2t-1)+QK(2t+1)
            AV runs nh-halves sequentially so each PSUM accumulator frees
            while the other half streams; QK m-chunks spread across both.
  tail      AV(10), AV(11), then proj with the last outcT chunk (c=5)
            accumulated last, evicted per 512/256-col leg to pipeline DMA.
"""

import os
from contextlib import ExitStack

import numpy as np

B, N, C = 8, 1024, 768
H, D = 12, 64
SCALE = D**-0.5
NCORES = 8

P = 128
CC = C // P        # 6  c-chunks
NT = N // P        # 8  n-chunks of 128
VW = H * (D + 1)   # 780: v + ones column per head

MM_MODE = os.environ.get("BASS_MM", "bf16")
PT_BUFS = int(os.environ.get("BASS_PT_BUFS", "24"))

_built = {}


def _build():
    import concourse.bass as bass  # noqa: F401
    import concourse.mybir as mybir
    import concourse.tile as tile
    from concourse import bacc

    f32 = mybir.dt.float32
    fmm = {"bf16": mybir.dt.bfloat16, "f32r": mybir.dt.float32r}[MM_MODE]
    AF = mybir.ActivationFunctionType
    ALU = mybir.AluOpType

    nc = bacc.Bacc("TRN2", target_bir_lowering=False, debug=False, num_devices=NCORES)

    xT_d = nc.dram_tensor("xT", [C, N], fmm, kind="ExternalInput").ap()
    wqk_d = nc.dram_tensor("wqk", [C, 2 * C], fmm, kind="ExternalInput").ap()
    wv_d = nc.dram_tensor("wv", [C, C], fmm, kind="ExternalInput").ap()
    wpT_d = nc.dram_tensor("wpT", [C, C], fmm, kind="ExternalInput").ap()
    bq_d = nc.dram_tensor("bq", [P, CC], f32, kind="ExternalInput").ap()
    pbe_d = nc.dram_tensor("pbe", [P, C], f32, kind="ExternalInput").ap()
    out_d = nc.dram_tensor("out", [N, C], f32, kind="ExternalOutput").ap()

    with tile.TileContext(nc) as tc, ExitStack() as ctx:
        persist = ctx.enter_context(tc.tile_pool(name="persist", bufs=1))
        qkpool = ctx.enter_context(tc.tile_pool(name="qkpool", bufs=1))
        rpool = ctx.enter_context(tc.tile_pool(name="rpool", bufs=4))
        ppool = ctx.enter_context(tc.tile_pool(name="ppool", bufs=PT_BUFS))
        wqpool = ctx.enter_context(tc.tile_pool(name="wqpool", bufs=2))
        ocpool = ctx.enter_context(tc.tile_pool(name="ocpool", bufs=1))
        ostage = ctx.enter_context(tc.tile_pool(name="ostage", bufs=4))
        ps2 = ctx.enter_context(tc.tile_pool(name="ps2", bufs=3, space="PSUM"))
        psav = ctx.enter_context(tc.tile_pool(name="psav", bufs=2, space="PSUM"))

        x_all = persist.tile([P, CC * N], fmm, name="x_all", tag="x_all")
        xv = x_all.rearrange("p (c n) -> p c n", n=N)
        vext = [
            persist.tile([P, VW], fmm, name=f"vext{i}", tag=f"vext{i}")
            for i in range(NT)
        ]
        bq_t = persist.tile([P, CC], f32, name="bq_t", tag="bq_t")
        wv_all = persist.tile([P, CC * C], fmm, name="wv_all", tag="wv_all")
        wvv = wv_all.rearrange("p (c f) -> p c f", f=C)
        wp_all = persist.tile([P, CC * C], fmm, name="wp_all", tag="wp_all")
        wpv = wp_all.rearrange("p (c f) -> p c f", f=C)
        pbe_t = persist.tile([P, C], f32, name="pbe_t", tag="pbe_t")
        outcT = [
            ocpool.tile([P, N], fmm, name=f"outcT{i}", tag=f"outcT{i}")
            for i in range(CC)
        ]

        # ---------------- startup DMAs ----------------
        # SWDGE (Pool) stream: bq, wq(0) — parallel to the HWDGE stream.
        nc.gpsimd.dma_start(bq_t[:], bq_d[:])

        def load_wq(t, eng):
            wq = wqpool.tile([P, CC * 2 * P], fmm, name="wq", tag="wq")
            eng.dma_start(
                wq.rearrange("p (c w) -> p c w", w=2 * P),
                wqk_d[:, t * 2 * P : (t + 1) * 2 * P].rearrange(
                    "(c p) w -> p c w", p=P
                ),
            )
            return wq.rearrange("p (c w) -> p c w", w=2 * P)

        wq0 = load_wq(0, nc.gpsimd)

        # HWDGE (SP) stream, priority order: x first halves, x second halves,
        # then wv (first needed ~7us in).
        xsrc = xT_d.rearrange("(c p) n -> p c n", p=P)
        for c0, c1 in ((0, 3), (3, 6)):
            nc.sync.dma_start(xv[:, c0:c1, 0:512], xsrc[:, c0:c1, 0:512])
        for c0, c1 in ((0, 3), (3, 6)):
            nc.sync.dma_start(xv[:, c0:c1, 512:1024], xsrc[:, c0:c1, 512:1024])
        wvsrc = wv_d.rearrange("(c p) f -> p c f", p=P)
        for c0, c1 in ((0, 3), (3, 6)):
            nc.sync.dma_start(wvv[:, c0:c1, :], wvsrc[:, c0:c1, :])

        for i in range(NT):
            nc.vector.memset(
                vext[i].rearrange("p (h e) -> p h e", e=D + 1)[:, :, D : D + 1],
                1.0,
            )

        pT = {}  # (h, mc) -> tile

        def emit_qk(t, wq=None):
            """Produce qT/kT ring tiles for heads 2t, 2t+1 (streams weights).
            nh-outer order so each n-half is evicted as soon as both psum
            tiles have it, letting scores start early."""
            if wq is None:
                wq = load_wq(t, nc.gpsimd)
            qT_t = qkpool.tile([P, N], fmm, name="qT_t", tag="qT_t")
            kT_t = qkpool.tile([P, N], fmm, name="kT_t", tag="kT_t")
            pss = [ps2.tile([P, N], f32, name="ps", tag="ps") for _ in range(2)]
            for nh in range(2):
                s = slice(nh * 512, (nh + 1) * 512)
                for wofs in range(2):
                    for c in range(CC):
                        nc.tensor.matmul(
                            pss[wofs][:, s],
                            wq[:, c, wofs * P : (wofs + 1) * P],
                            xv[:, c, s],
                            start=(c == 0),
                            stop=(c == CC - 1),
                        )
                nc.vector.tensor_scalar(
                    out=qT_t[:, s],
                    in0=pss[0][:, s],
                    scalar1=SCALE,
                    scalar2=bq_t[:, t : t + 1],
                    op0=ALU.mult,
                    op1=ALU.add,
                )
                nc.vector.tensor_copy(kT_t[:, s], pss[1][:, s])
            return qT_t, kT_t

        def emit_QK_mc(h, mc, qT_t, kT_t):
            """One m-chunk of scores for head h: S.T[mc-block, :] -> exp -> pT."""
            r0 = (h % 2) * D
            ps = ps2.tile([P, N], f32, name="ps", tag="ps")
            for nh in range(2):
                nc.tensor.matmul(
                    ps[:, nh * 512 : (nh + 1) * 512],
                    kT_t[r0 : r0 + D, mc * P : (mc + 1) * P],
                    qT_t[r0 : r0 + D, nh * 512 : (nh + 1) * 512],
                    start=True,
                    stop=True,
                )
            pt = ppool.tile([P, N], fmm, name="pt", tag="pt")
            nc.scalar.activation(pt[:], ps[:], AF.Exp)
            pT[(h, mc)] = pt

        def emit_AV(h, interleave=None):
            """Attention @ V for head h, nh-halves sequential so each PSUM
            accumulator is freed (normalize chain) while the other half
            streams. QK m-chunks of a later head are spread across both
            halves to keep the ACT pipeline fed evenly."""
            ti, r0 = h // 2, (h % 2) * D
            psa = [
                psav.tile([D + 1, 512], f32, name="psa", tag="psa") for _ in range(2)
            ]
            for nh in range(2):
                for mc in range(NT):
                    nc.tensor.matmul(
                        psa[nh][:],
                        vext[mc][:, h * (D + 1) : (h + 1) * (D + 1)],
                        pT[(h, mc)][:, nh * 512 : (nh + 1) * 512],
                        start=(mc == 0),
                        stop=(mc == NT - 1),
                    )
                    if interleave is not None and mc % 2 == 0:
                        h2, qT_t, kT_t = interleave
                        emit_QK_mc(h2, nh * (NT // 2) + mc // 2, qT_t, kT_t)
                # normalize: 1/Z straight off the PSUM Z-row (partition 64)
                # into SBUF partition 0, broadcast, multiply from PSUM
                rc = rpool.tile([1, 512], f32, name="rc", tag="rc")
                nc.vector.reciprocal(rc[:], psa[nh][D : D + 1, :])
                rb = rpool.tile([D, 512], f32, name="rb", tag="rb")
                nc.gpsimd.partition_broadcast(rb[:], rc[:])
                nc.vector.tensor_tensor(
                    out=outcT[ti][r0 : r0 + D, nh * 512 : (nh + 1) * 512],
                    in0=psa[nh][0:D, :],
                    in1=rb[:],
                    op=ALU.mult,
                )
            for mc in range(NT):
                del pT[(h, mc)]

        # ---------------- prelude ----------------
        qT_t, kT_t = emit_qk(0, wq=wq0)
        vps = {}
        for mc in range(NT):
            emit_QK_mc(0, mc, qT_t, kT_t)
            # v-projection leg A (features 0:512)
            vps[mc] = ps2.tile([P, C], f32, name="ps", tag="ps")
            for c in range(CC):
                nc.tensor.matmul(
                    vps[mc][:, 0:512],
                    xv[:, c, mc * P : (mc + 1) * P],
                    wvv[:, c, 0:512],
                    start=(c == 0),
                    stop=(c == CC - 1),
                )
            emit_QK_mc(1, mc, qT_t, kT_t)
            # v-projection leg B (features 512:768) + eviction into vext
            for c in range(CC):
                nc.tensor.matmul(
                    vps[mc][:, 512:768],
                    xv[:, c, mc * P : (mc + 1) * P],
                    wvv[:, c, 512:768],
                    start=(c == 0),
                    stop=(c == CC - 1),
                )
            nc.vector.tensor_copy(
                vext[mc].rearrange("p (h e) -> p h e", e=D + 1)[:, :, 0:D],
                vps[mc].rearrange("p (h d) -> p h d", d=D),
            )
            del vps[mc]

        # proj weights + bias: fetch during steady state (DMA has slack there)
        wpsrc = wpT_d.rearrange("(c p) f -> p c f", p=P)
        for c0, c1 in ((0, 3), (3, 6)):
            nc.sync.dma_start(wpv[:, c0:c1, :], wpsrc[:, c0:c1, :])
        nc.sync.dma_start(pbe_t[:], pbe_d[:])

        # ---------------- steady state ----------------
        for t in range(1, CC):
            qT_n, kT_n = emit_qk(t)
            emit_AV(2 * t - 2, interleave=(2 * t, qT_n, kT_n))
            emit_AV(2 * t - 1, interleave=(2 * t + 1, qT_n, kT_n))
            qT_t, kT_t = qT_n, kT_n
        emit_AV(2 * CC - 2)
        emit_AV(2 * CC - 1)

        # ---------------- output projection ----------------
        for nt in range(NT):
            ps = ps2.tile([P, C], f32, name="ps", tag="ps")
            for o0, o1 in ((0, 512), (512, 768)):
                for c in range(CC - 1):
                    nc.tensor.matmul(
                        ps[:, o0:o1],
                        outcT[c][:, nt * P : (nt + 1) * P],
                        wpv[:, c, o0:o1],
                        start=(c == 0),
                        stop=False,
                    )
            # last chunk (heads 10/11) accumulated last: its eviction chain
            # finishes while the earlier chunks stream
            for o0, o1 in ((0, 512), (512, 768)):
                nc.tensor.matmul(
                    ps[:, o0:o1],
                    outcT[CC - 1][:, nt * P : (nt + 1) * P],
                    wpv[:, CC - 1, o0:o1],
                    start=False,
                    stop=True,
                )
            for o0, o1 in ((0, 512), (512, 768)):
                ot = ostage.tile([P, 512], f32, name="ot", tag="ot")
                nc.vector.tensor_add(ot[:, 0 : o1 - o0], ps[:, o0:o1], pbe_t[:, o0:o1])
                nc.sync.dma_start(
                    out_d[nt * P : (nt + 1) * P, o0:o1], ot[:, 0 : o1 - o0]
                )

    nc.compile()
    return nc


def kernel(x, qkv_w, qkv_b, proj_w, proj_b):
    from concourse.bass_utils import run_bass_kernel_spmd

    key = (MM_MODE, PT_BUFS)
    if key not in _built:
        _built[key] = _build()
    nc = _built[key]

    x = np.asarray(x, np.float32)
    qkv_w = np.asarray(qkv_w, np.float32)
    qkv_b = np.asarray(qkv_b, np.float32)
    proj_w = np.asarray(proj_w, np.float32)
    proj_b = np.asarray(proj_b, np.float32)

    if MM_MODE == "bf16":
        import ml_dtypes

        mmdt = ml_dtypes.bfloat16
    else:
        mmdt = np.float32

    wT = np.ascontiguousarray(qkv_w.T)  # [C, 3C]
    # per-t interleave: block t = [q cols t*128:(t+1)*128 | k cols same range]
    wqk = np.concatenate(
        [
            np.concatenate(
                (wT[:, t * P : (t + 1) * P], wT[:, C + t * P : C + (t + 1) * P]),
                axis=1,
            )
            for t in range(CC)
        ],
        axis=1,
    )
    wqk = np.ascontiguousarray(wqk).astype(mmdt)
    wv = np.ascontiguousarray(wT[:, 2 * C :]).astype(mmdt)
    wpT = np.ascontiguousarray(proj_w.T).astype(mmdt)
    bq = np.ascontiguousarray((SCALE * qkv_b[:C]).reshape(CC, P).T)
    pbe = proj_b + qkv_b[2 * C :] @ proj_w.T
    pbe_b = np.ascontiguousarray(np.broadcast_to(pbe, (P, C)))

    in_maps = [
        {
            "xT": np.ascontiguousarray(x[b].T).astype(mmdt),
            "wqk": wqk,
            "wv": wv,
            "wpT": wpT,
            "bq": bq,
            "pbe": pbe_b,
        }
        for b in range(B)
    ]

    trace = bool(int(os.environ.get("BASS_PROFILE", "0")))
    res = run_bass_kernel_spmd(nc, in_maps, list(range(NCORES)), trace=trace)
    return np.stack([res.results[b]["out"] for b in range(B)])


# revision 7
# speedup vs baseline: 1.1298x; 1.0699x over previous
"""Multi-head self-attention (B=8, N=1024, C=768, H=12) on 8 Trainium2 cores.

Strategy: data parallel — one batch element per NeuronCore, no collectives.

Per-core program (x_b is [N, C], shipped pre-transposed as xT [C, N], all
matmul operands in bf16, PSUM accumulation in fp32):
  1. qkT  [o, n] = wqk[:, o].T @ xT            o in [0, 1536)   (q and k, transposed)
       q rows evicted with  *SCALE and +SCALE*bq  (k bias cancels in softmax)
  2. v    [n, o] = xT[:, n].T @ wv             (natural layout), evicted into
       vext [n, 12*65] with a ones column appended per head
  3. per head h:  S.T[m, n] = kT_h.T @ qT_h    (K=64 matmul)
       E = exp(S.T)  (ACT, no max subtraction: logits ~ N(0,1))
       U [65, n] = vext_h.T @ E  — rows 0..63 = unnormalized out.T, row 64 = Z
       1/Z via DVE reciprocal off the PSUM Z-row, gpsimd partition_broadcast,
       outcT_h [d, n] = U[0:64] * (1/Z)b  (DVE, straight from PSUM)
  4. final [n, co] = outcT[:, n].T @ wpT + pbe   (pbe = proj_b + bv @ proj_w.T)

Schedule: PE is the bottleneck (~144 us of matmul columns); everything else
is paced to keep it busy.
  - The 96 score-exp chunks (one ACT instruction each, 1038 ns) are the
    second-largest load (~100 us). They are spread uniformly via a work
    queue: 5 chunks interleave into each qk(t) phase (reading the PREVIOUS
    iteration's qT/kT — qkpool is double-buffered for this), 6 into each
    even-AV phase, 5 into each odd-AV phase, matching ACT throughput to PE
    phase time everywhere so the 3-buffer PSUM rotation never stalls PE.
  - AV runs its two n-halves sequentially so each [65,512] PSUM accumulator
    frees (normalize chain) while the other half streams.
  - Startup DMAs are fine-grained and spread across the SP/ACT/DVE HWDGE
    queues in consumption order (wq0 first), so qk(0) starts at ~2.5 us.
  - The tail interleaves proj partials for the first n-chunks into AV(11);
    the last outcT chunk (c=5) is accumulated last within each proj n-chunk.
"""

import os
from contextlib import ExitStack

import numpy as np

B, N, C = 8, 1024, 768
H, D = 12, 64
SCALE = D**-0.5
NCORES = 8

P = 128
CC = C // P        # 6  c-chunks
NT = N // P        # 8  n-chunks of 128
VW = H * (D + 1)   # 780: v + ones column per head

MM_MODE = os.environ.get("BASS_MM", "bf16")
PT_BUFS = int(os.environ.get("BASS_PT_BUFS", "24"))

_built = {}


def _build():
    import concourse.bass as bass  # noqa: F401
    import concourse.mybir as mybir
    import concourse.tile as tile
    from concourse import bacc

    f32 = mybir.dt.float32
    fmm = {"bf16": mybir.dt.bfloat16, "f32r": mybir.dt.float32r}[MM_MODE]
    AF = mybir.ActivationFunctionType
    ALU = mybir.AluOpType

    nc = bacc.Bacc("TRN2", target_bir_lowering=False, debug=False, num_devices=NCORES)

    xT_d = nc.dram_tensor("xT", [C, N], fmm, kind="ExternalInput").ap()
    wqk_d = nc.dram_tensor("wqk", [C, 2 * C], fmm, kind="ExternalInput").ap()
    wv_d = nc.dram_tensor("wv", [C, C], fmm, kind="ExternalInput").ap()
    wpT_d = nc.dram_tensor("wpT", [C, C], fmm, kind="ExternalInput").ap()
    bq_d = nc.dram_tensor("bq", [P, CC], f32, kind="ExternalInput").ap()
    pbe_d = nc.dram_tensor("pbe", [P, C], f32, kind="ExternalInput").ap()
    out_d = nc.dram_tensor("out", [N, C], f32, kind="ExternalOutput").ap()

    with tile.TileContext(nc) as tc, ExitStack() as ctx:
        persist = ctx.enter_context(tc.tile_pool(name="persist", bufs=1))
        qkpool = ctx.enter_context(tc.tile_pool(name="qkpool", bufs=2))
        rpool = ctx.enter_context(tc.tile_pool(name="rpool", bufs=4))
        ppool = ctx.enter_context(tc.tile_pool(name="ppool", bufs=PT_BUFS))
        wqpool = ctx.enter_context(tc.tile_pool(name="wqpool", bufs=2))
        ocpool = ctx.enter_context(tc.tile_pool(name="ocpool", bufs=1))
        ostage = ctx.enter_context(tc.tile_pool(name="ostage", bufs=4))
        ps2 = ctx.enter_context(tc.tile_pool(name="ps2", bufs=3, space="PSUM"))
        psav = ctx.enter_context(tc.tile_pool(name="psav", bufs=2, space="PSUM"))

        x_all = persist.tile([P, CC * N], fmm, name="x_all", tag="x_all")
        xv = x_all.rearrange("p (c n) -> p c n", n=N)
        vext = [
            persist.tile([P, VW], fmm, name=f"vext{i}", tag=f"vext{i}")
            for i in range(NT)
        ]
        bq_t = persist.tile([P, CC], f32, name="bq_t", tag="bq_t")
        wv_all = persist.tile([P, CC * C], fmm, name="wv_all", tag="wv_all")
        wvv = wv_all.rearrange("p (c f) -> p c f", f=C)
        wp_all = persist.tile([P, CC * C], fmm, name="wp_all", tag="wp_all")
        wpv = wp_all.rearrange("p (c f) -> p c f", f=C)
        pbe_t = persist.tile([P, C], f32, name="pbe_t", tag="pbe_t")
        outcT = [
            ocpool.tile([P, N], fmm, name=f"outcT{i}", tag=f"outcT{i}")
            for i in range(CC)
        ]

        # ---------------- startup DMAs ----------------
        # Fine-grained, spread across three HWDGE-issuing engines in
        # consumption order: wq0 halves first, then x first halves, x second
        # halves, wv. bq rides the DVE queue (tiny, needed ~8 us in).
        wq0 = wqpool.tile([P, CC * 2 * P], fmm, name="wq", tag="wq")
        wq0v = wq0.rearrange("p (c w) -> p c w", w=2 * P)
        wqsrc = wqk_d.rearrange("(c p) w -> p c w", p=P)
        xsrc = xT_d.rearrange("(c p) n -> p c n", p=P)
        wvsrc = wv_d.rearrange("(c p) f -> p c f", p=P)

        nc.sync.dma_start(wq0v[:, :, 0:P], wqsrc[:, :, 0:P])
        nc.scalar.dma_start(wq0v[:, :, P : 2 * P], wqsrc[:, :, P : 2 * P])
        nc.gpsimd.dma_start(bq_t[:], bq_d[:])
        for c in range(CC):
            eng = nc.sync if c % 2 == 0 else nc.scalar
            eng.dma_start(xv[:, c, 0:512], xsrc[:, c, 0:512])
        for c in range(CC):
            eng = nc.sync if c % 2 == 0 else nc.scalar
            eng.dma_start(xv[:, c, 512:1024], xsrc[:, c, 512:1024])
        for c0 in range(0, CC, 2):
            eng = nc.sync if c0 % 4 == 0 else nc.scalar
            eng.dma_start(wvv[:, c0 : c0 + 2, :], wvsrc[:, c0 : c0 + 2, :])

        for i in range(NT):
            nc.vector.memset(
                vext[i].rearrange("p (h e) -> p h e", e=D + 1)[:, :, D : D + 1],
                1.0,
            )

        pT = {}        # (h, mc) -> tile
        qk_queue = []  # pending score chunks: (h, mc, qT_t, kT_t)

        def pop_score(n=1):
            for _ in range(n):
                if not qk_queue:
                    return
                h, mc, qT_t, kT_t = qk_queue.pop(0)
                r0 = (h % 2) * D
                ps = ps2.tile([P, N], f32, name="ps", tag="ps")
                for nh in range(2):
                    nc.tensor.matmul(
                        ps[:, nh * 512 : (nh + 1) * 512],
                        kT_t[r0 : r0 + D, mc * P : (mc + 1) * P],
                        qT_t[r0 : r0 + D, nh * 512 : (nh + 1) * 512],
                        start=True,
                        stop=True,
                    )
                pt = ppool.tile([P, N], fmm, name="pt", tag="pt")
                nc.scalar.activation(pt[:], ps[:], AF.Exp)
                pT[(h, mc)] = pt

        def load_wq(t):
            wq = wqpool.tile([P, CC * 2 * P], fmm, name="wq", tag="wq")
            nc.gpsimd.dma_start(
                wq.rearrange("p (c w) -> p c w", w=2 * P),
                wqsrc[:, :, t * 2 * P : (t + 1) * 2 * P],
            )
            return wq.rearrange("p (c w) -> p c w", w=2 * P)

        def emit_qk(t, wq=None, fills=(0, 0, 0, 0)):
            """qkv projection for heads 2t, 2t+1. nh-outer so each n-half is
            evicted as soon as both psum tiles have it. fills = score chunks
            to interleave after each (nh, wofs) matmul group."""
            if wq is None:
                wq = load_wq(t)
            qT_t = qkpool.tile([P, N], fmm, name="qT_t", tag="qT_t")
            kT_t = qkpool.tile([P, N], fmm, name="kT_t", tag="kT_t")
            pss = [ps2.tile([P, N], f32, name="ps", tag="ps") for _ in range(2)]
            g = 0
            for nh in range(2):
                s = slice(nh * 512, (nh + 1) * 512)
                for wofs in range(2):
                    for c in range(CC):
                        nc.tensor.matmul(
                            pss[wofs][:, s],
                            wq[:, c, wofs * P : (wofs + 1) * P],
                            xv[:, c, s],
                            start=(c == 0),
                            stop=(c == CC - 1),
                        )
                    pop_score(fills[g])
                    g += 1
                nc.vector.tensor_scalar(
                    out=qT_t[:, s],
                    in0=pss[0][:, s],
                    scalar1=SCALE,
                    scalar2=bq_t[:, t : t + 1],
                    op0=ALU.mult,
                    op1=ALU.add,
                )
                nc.vector.tensor_copy(kT_t[:, s], pss[1][:, s])
            # queue this head-pair's score work (consumed over the next phases)
            for h in (2 * t, 2 * t + 1):
                for mc in range(NT):
                    qk_queue.append((h, mc, qT_t, kT_t))
            return qT_t, kT_t

        def emit_AV(h, n_fill, fillers=None):
            """Attention @ V for head h, nh-halves sequential so each PSUM
            accumulator frees (normalize chain) while the other streams.
            n_fill score chunks (or explicit filler thunks) interleave."""
            ti, r0 = h // 2, (h % 2) * D
            psa = [
                psav.tile([D + 1, 512], f32, name="psa", tag="psa") for _ in range(2)
            ]
            nf = 0
            for nh in range(2):
                for mc in range(NT):
                    nc.tensor.matmul(
                        psa[nh][:],
                        vext[mc][:, h * (D + 1) : (h + 1) * (D + 1)],
                        pT[(h, mc)][:, nh * 512 : (nh + 1) * 512],
                        start=(mc == 0),
                        stop=(mc == NT - 1),
                    )
                    want = (nf + 1) * 2 * NT <= (nh * NT + mc + 1) * n_fill
                    if want and nf < n_fill:
                        if fillers is not None:
                            fillers[nf]()
                        else:
                            pop_score(1)
                        nf += 1
                # normalize: 1/Z off the PSUM Z-row into SBUF partition 0,
                # broadcast, multiply straight from PSUM
                rc = rpool.tile([1, 512], f32, name="rc", tag="rc")
                nc.vector.reciprocal(rc[:], psa[nh][D : D + 1, :])
                rb = rpool.tile([D, 512], f32, name="rb", tag="rb")
                nc.gpsimd.partition_broadcast(rb[:], rc[:])
                nc.vector.tensor_tensor(
                    out=outcT[ti][r0 : r0 + D, nh * 512 : (nh + 1) * 512],
                    in0=psa[nh][0:D, :],
                    in1=rb[:],
                    op=ALU.mult,
                )
            for mc in range(NT):
                del pT[(h, mc)]

        # ---------------- prelude: qk(0), scores(0,·)+(1,0..2) ⊗ v ----------
        qT_t, kT_t = emit_qk(0, wq=wq0v)
        for it in range(NT):
            pop_score(1)
            ps_v = ps2.tile([P, C], f32, name="ps", tag="ps")
            for c in range(CC):
                nc.tensor.matmul(
                    ps_v[:, 0:512],
                    xv[:, c, it * P : (it + 1) * P],
                    wvv[:, c, 0:512],
                    start=(c == 0),
                    stop=(c == CC - 1),
                )
            if it % 2 == 0 and it < 6:
                pop_score(1)
            for c in range(CC):
                nc.tensor.matmul(
                    ps_v[:, 512:768],
                    xv[:, c, it * P : (it + 1) * P],
                    wvv[:, c, 512:768],
                    start=(c == 0),
                    stop=(c == CC - 1),
                )
            nc.vector.tensor_copy(
                vext[it].rearrange("p (h e) -> p h e", e=D + 1)[:, :, 0:D],
                ps_v.rearrange("p (h d) -> p h d", d=D),
            )
        # 11 chunks consumed: (0,0..7) and (1,0..2); (1,3..7) stay queued.

        # proj weights + bias: fetched during steady state (DMA slack there)
        wpsrc = wpT_d.rearrange("(c p) f -> p c f", p=P)
        for c0, c1 in ((0, 3), (3, 6)):
            nc.sync.dma_start(wpv[:, c0:c1, :], wpsrc[:, c0:c1, :])
        nc.sync.dma_start(pbe_t[:], pbe_d[:])

        # ---------------- steady state ----------------
        for t in range(1, CC):
            emit_qk(t, fills=(1, 1, 1, 2))
            emit_AV(2 * t - 2, n_fill=6)
            emit_AV(2 * t - 1, n_fill=5)

        # ---------------- tail: AV(10) ⊗ (11,3..7); AV(11) ⊗ proj nt=0 ------
        emit_AV(2 * CC - 2, n_fill=5)

        proj_ps = {}

        def proj_partial(nt, o0, o1, cs, start, stop):
            def thunk():
                if nt not in proj_ps:
                    proj_ps[nt] = ps2.tile([P, C], f32, name="ps", tag="ps")
                for c in cs:
                    nc.tensor.matmul(
                        proj_ps[nt][:, o0:o1],
                        outcT[c][:, nt * P : (nt + 1) * P],
                        wpv[:, c, o0:o1],
                        start=(c == cs[0]) and start,
                        stop=(c == cs[-1]) and stop,
                    )
            return thunk

        c04 = list(range(CC - 1))
        emit_AV(
            2 * CC - 1,
            n_fill=4,
            fillers=[
                proj_partial(0, 0, 512, c04[:3], True, False),
                proj_partial(0, 0, 512, c04[3:], False, False),
                proj_partial(0, 512, 768, c04[:3], True, False),
                proj_partial(0, 512, 768, c04[3:], False, False),
            ],
        )

        def proj_evict(nt, o0, o1):
            ot = ostage.tile([P, 512], f32, name="ot", tag="ot")
            nc.vector.tensor_add(
                ot[:, 0 : o1 - o0], proj_ps[nt][:, o0:o1], pbe_t[:, o0:o1]
            )
            nc.sync.dma_start(out_d[nt * P : (nt + 1) * P, o0:o1], ot[:, 0 : o1 - o0])

        # finish nt=0 (c=5 legs), then nt=1..7 with c=5 accumulated last
        for o0, o1 in ((0, 512), (512, 768)):
            proj_partial(0, o0, o1, [CC - 1], False, True)()
        for o0, o1 in ((0, 512), (512, 768)):
            proj_evict(0, o0, o1)
        for nt in range(1, NT):
            for o0, o1 in ((0, 512), (512, 768)):
                proj_partial(nt, o0, o1, c04, True, False)()
            for o0, o1 in ((0, 512), (512, 768)):
                proj_partial(nt, o0, o1, [CC - 1], False, True)()
            for o0, o1 in ((0, 512), (512, 768)):
                proj_evict(nt, o0, o1)

    nc.compile()
    return nc


def kernel(x, qkv_w, qkv_b, proj_w, proj_b):
    from concourse.bass_utils import run_bass_kernel_spmd

    key = (MM_MODE, PT_BUFS)
    if key not in _built:
        _built[key] = _build()
    nc = _built[key]

    x = np.asarray(x, np.float32)
    qkv_w = np.asarray(qkv_w, np.float32)
    qkv_b = np.asarray(qkv_b, np.float32)
    proj_w = np.asarray(proj_w, np.float32)
    proj_b = np.asarray(proj_b, np.float32)

    if MM_MODE == "bf16":
        import ml_dtypes

        mmdt = ml_dtypes.bfloat16
    else:
        mmdt = np.float32

    wT = np.ascontiguousarray(qkv_w.T)  # [C, 3C]
    # per-t interleave: block t = [q cols t*128:(t+1)*128 | k cols same range]
    wqk = np.concatenate(
        [
            np.concatenate(
                (wT[:, t * P : (t + 1) * P], wT[:, C + t * P : C + (t + 1) * P]),
                axis=1,
            )
            for t in range(CC)
        ],
        axis=1,
    )
    wqk = np.ascontiguousarray(wqk).astype(mmdt)
    wv = np.ascontiguousarray(wT[:, 2 * C :]).astype(mmdt)
    wpT = np.ascontiguousarray(proj_w.T).astype(mmdt)
    bq = np.ascontiguousarray((SCALE * qkv_b[:C]).reshape(CC, P).T)
    pbe = proj_b + qkv_b[2 * C :] @ proj_w.T
    pbe_b = np.ascontiguousarray(np.broadcast_to(pbe, (P, C)))

    in_maps = [
        {
            "xT": np.ascontiguousarray(x[b].T).astype(mmdt),
            "wqk": wqk,
            "wv": wv,
            "wpT": wpT,
            "bq": bq,
            "pbe": pbe_b,
        }
        for b in range(B)
    ]

    trace = bool(int(os.environ.get("BASS_PROFILE", "0")))
    res = run_bass_kernel_spmd(nc, in_maps, list(range(NCORES)), trace=trace)
    return np.stack([res.results[b]["out"] for b in range(B)])
